# revision 11
# baseline (speedup 1.0000x reference)
"""COMA loss kernel for Trainium2 (8 NeuronCores, data-parallel over batch).

Reference computation: given logit/q_value/target_q_value (T,B,A,N),
action (T,B,A), reward (T,B), weight (T,B,A), compute
(policy_loss, q_value_loss, entropy_loss) scalars.

Sharding: B=128 split 8 ways -> B_local=16 per core; the (b,a) plane is
16*8 = 128 rows = the SBUF partition count. Per-core layout is
[BA=128, T, ...], all big tensors cast to bf16 on the host (rel-err
impact measured at ~3e-4 on CPU, far under the 2e-2 gate).

Perf notes (measured on HW, not the cost model):
  * tensor_tensor bf16 SBUF = 2x mode (0.54 ns/elem); scalar_tensor_tensor
    has NO fast uop (1x) and tensor_reduce is 1x -- so all elementwise ops
    are plain tensor_tensor and every per-(t,ba) sum over N is a pairwise
    halving tree of TT adds (2x), merged across the 6 reduced quantities
    into one 6-wide tree to amortize per-op overhead.
  * exp and the onehot comparand broadcast run on the Scalar engine.
  * the three input tensors are packed host-side into one [BA,T,3N] tensor;
    chunk DMAs are split across the SP, Activation and GpSimd(SWDGE) queues
    (one queue sustains only ~114 GB/s; three run concurrently).

Each core emits per-partition partial sums [128,3]; the host adds them
and divides by the global element counts (the all-reduce of the three
scalar means).
"""

import sys

for _p in ("/opt/trn_rl_repo",):
    if _p not in sys.path:
        sys.path.insert(0, _p)

import numpy as np
import ml_dtypes

import concourse.bass as bass
import concourse.bacc as bacc
import concourse.mybir as mybir
from concourse.bass_utils import run_bass_kernel_spmd
from concourse.tile import TileContext

T, B, A, N = 256, 128, 8, 64
M = 8                 # cores
BL = B // M           # local batch
BA = BL * A           # 128 rows -> partition dim
TC = 64               # t-chunk size
NCH = T // TC
GAMMA, LAMBDA = 0.99, 0.95

F32 = mybir.dt.float32
BF16 = mybir.dt.bfloat16
BF16_NP = ml_dtypes.bfloat16

# merged-reduce group order within the 6*N-wide product tile
J_LTK, J_QTK, J_TQTK, J_EL, J_EQ, J_SE = range(6)


def build_program() -> bass.Bass:
    nc = bacc.Bacc("TRN2", target_bir_lowering=False, debug=False)

    # packed [lg | qt | tq] along the last axis
    vd = nc.dram_tensor("v", [BA, T, 3 * N], BF16, kind="ExternalInput")
    act = nc.dram_tensor("act", [BA, T], BF16, kind="ExternalInput")
    wgt = nc.dram_tensor("wgt", [BA, T], F32, kind="ExternalInput")
    rwd = nc.dram_tensor("rwd", [BA, T], F32, kind="ExternalInput")
    out = nc.dram_tensor("out", [BA, 3], F32, kind="ExternalOutput")

    OP = mybir.AluOpType
    AX = mybir.AxisListType
    Act = mybir.ActivationFunctionType

    with TileContext(nc) as tc:
        with (
            tc.tile_pool(name="inp", bufs=2) as inp,
            tc.tile_pool(name="grpp", bufs=1) as grpp,
            tc.tile_pool(name="scr", bufs=1) as scr,
            tc.tile_pool(name="per", bufs=1) as per,
        ):
            # iota over N as bf16 (0..63 exact)
            iota_i = per.tile([BA, N], mybir.dt.int32)
            nc.gpsimd.iota(iota_i[:], pattern=[[1, N]], base=0, channel_multiplier=0)
            iota_f = per.tile([BA, N], BF16)
            nc.vector.tensor_copy(iota_f[:], iota_i[:])

            act_t = per.tile([BA, T], BF16)
            nc.sync.dma_start(out=act_t[:], in_=act[:])
            w_t = per.tile([BA, T], F32)
            nc.sync.dma_start(out=w_t[:], in_=wgt[:])
            r_t = per.tile([BA, T], F32)
            nc.sync.dma_start(out=r_t[:], in_=rwd[:])

            # the 6 per-(t,ba) reductions, interleaved: [BA, T, 6]
            acc6 = per.tile([BA, T, 6], F32)

            for c in range(NCH):
                t0 = c * TC
                sl = slice(t0, t0 + TC)

                v = inp.tile([BA, TC, 3 * N], BF16, tag="v")
                # split each chunk across the three DMA queues
                s1, s2 = TC // 3, 2 * TC // 3
                nc.sync.dma_start(
                    out=v[:, 0:s1], in_=vd[:, t0 : t0 + s1]
                )
                nc.scalar.dma_start(
                    out=v[:, s1:s2], in_=vd[:, t0 + s1 : t0 + s2]
                )
                nc.gpsimd.dma_start(
                    out=v[:, s2:TC], in_=vd[:, t0 + s2 : t0 + TC]
                )
                lg, qt, tq = (v[:, :, j * N : (j + 1) * N] for j in range(3))

                # onehot comparand: action replicated across N (Scalar engine)
                arep = scr.tile([BA, TC, N], BF16, tag="arep")
                nc.scalar.activation(
                    out=arep[:],
                    in_=act_t[:, sl].unsqueeze(2).to_broadcast([BA, TC, N]),
                    func=Act.Copy,
                )

                # product groups, contiguous so one merged tree reduces all 6:
                # [oh*lg | oh*qt | oh*tq | e*lg | e*qt | e]
                grp = grpp.tile([BA, TC, 6 * N], BF16, tag="grp")
                e = grp[:, :, J_SE * N : (J_SE + 1) * N]
                nc.scalar.activation(out=e, in_=lg, func=Act.Exp)

                oh = scr.tile([BA, TC, N], BF16, tag="oh")
                nc.vector.tensor_tensor(
                    out=oh[:],
                    in0=iota_f[:].unsqueeze(1).to_broadcast([BA, TC, N]),
                    in1=arep[:],
                    op=OP.is_equal,
                )

                for j, sec in ((J_LTK, lg), (J_QTK, qt)):
                    nc.vector.tensor_mul(
                        grp[:, :, j * N : (j + 1) * N], oh[:], sec
                    )
                # the tq gather product runs on the otherwise-idle GpSimd
                nc.gpsimd.tensor_mul(
                    grp[:, :, J_TQTK * N : (J_TQTK + 1) * N], oh[:], tq
                )
                nc.vector.tensor_mul(grp[:, :, J_EL * N : (J_EL + 1) * N], e, lg)
                nc.vector.tensor_mul(grp[:, :, J_EQ * N : (J_EQ + 1) * N], e, qt)

                # merged pairwise tree over the last-axis width N for all 6
                # groups at once; 4-D views [BA, TC, 6, w]
                cur, w = grp[:].rearrange("p t (g n) -> p t g n", g=6), N
                while w > 2:
                    h = w // 2
                    nxt = scr.tile([BA, TC, 6, h], BF16, tag=f"tr{h}")
                    nc.vector.tensor_add(
                        nxt[:], cur[:, :, :, 0:h], cur[:, :, :, h:w]
                    )
                    cur, w = nxt, h
                nc.vector.tensor_add(
                    acc6[:, sl].unsqueeze(3),
                    cur[:, :, :, 0:1],
                    cur[:, :, :, 1:2],
                )

            # ---- stage 2: per-(t,ba) scalar math on [128, T] -------------
            l_tk = acc6[:, :, J_LTK]
            q_tk = acc6[:, :, J_QTK]
            tq_tk = acc6[:, :, J_TQTK]
            dot_el = acc6[:, :, J_EL]
            dot_eq = acc6[:, :, J_EQ]
            sum_e = acc6[:, :, J_SE]

            z = per.tile([BA, T], F32)  # logsumexp
            nc.scalar.activation(out=z[:], in_=sum_e, func=Act.Ln)
            rs = per.tile([BA, T], F32)  # 1/sum_e
            nc.vector.reciprocal(rs[:], sum_e)

            logp = per.tile([BA, T], F32)
            nc.vector.tensor_tensor(out=logp[:], in0=l_tk, in1=z[:], op=OP.subtract)
            bl = per.tile([BA, T], F32)  # baseline = dot_eq / sum_e
            nc.vector.tensor_mul(bl[:], dot_eq, rs[:])
            adv = per.tile([BA, T], F32)
            nc.vector.tensor_tensor(out=adv[:], in0=q_tk, in1=bl[:], op=OP.subtract)
            ent = per.tile([BA, T], F32)  # entropy = z - dot_el / sum_e
            nc.vector.tensor_mul(ent[:], dot_el, rs[:])
            nc.vector.tensor_tensor(out=ent[:], in0=z[:], in1=ent[:], op=OP.subtract)

            pol = per.tile([BA, T], F32)  # logp * adv * w
            nc.vector.tensor_mul(pol[:], logp[:], adv[:])
            nc.vector.tensor_mul(pol[:], pol[:], w_t[:])
            entw = per.tile([BA, T], F32)
            nc.vector.tensor_mul(entw[:], ent[:], w_t[:])

            # lambda returns: ret[t] = d[t] + g*l*ret[t+1], scanned in
            # reverse time via negative-step views.
            # d[t] = reward[t] + g*(1-l)*tq_taken[t+1], t in [0, T-2];
            # initial state tq_taken[T-1] makes ret[T-2] = reward[T-2] +
            # g*tq_taken[T-1] as required.
            d = per.tile([BA, T - 1], F32)
            nc.vector.tensor_scalar_mul(d[:], tq_tk[:, 1:T], GAMMA * (1.0 - LAMBDA))
            nc.vector.tensor_add(d[:], d[:], r_t[:, 0 : T - 1])
            gl = per.tile([BA, 1], F32)
            nc.vector.memset(gl[:], GAMMA * LAMBDA)
            ret = per.tile([BA, T - 1], F32)
            nc.vector.tensor_tensor_scan(
                out=ret[:, ::-1],
                data0=gl[:].to_broadcast([BA, T - 1]),
                data1=d[:, ::-1],
                initial=tq_tk[:, T - 1 : T],
                op0=OP.mult,
                op1=OP.add,
            )

            qd = per.tile([BA, T - 1], F32)
            nc.vector.tensor_tensor(
                out=qd[:], in0=ret[:], in1=q_tk[:, 0 : T - 1], op=OP.subtract
            )
            nc.vector.tensor_mul(qd[:], qd[:], qd[:])
            nc.vector.tensor_mul(qd[:], qd[:], w_t[:, 0 : T - 1])

            partials = per.tile([BA, 3], F32)
            nc.vector.reduce_sum(out=partials[:, 0:1], in_=pol[:], axis=AX.X)
            nc.vector.reduce_sum(out=partials[:, 1:2], in_=qd[:], axis=AX.X)
            nc.vector.reduce_sum(out=partials[:, 2:3], in_=entw[:], axis=AX.X)
            nc.sync.dma_start(out=out[:], in_=partials[:])

    return nc


def make_in_maps(logit, action, q_value, target_q_value, reward, weight):
    """Shard + marshal full inputs into per-core input dicts."""
    logit = np.asarray(logit, np.float32)
    q_value = np.asarray(q_value, np.float32)
    target_q_value = np.asarray(target_q_value, np.float32)
    action = np.asarray(action)
    reward = np.asarray(reward, np.float32)
    weight = np.asarray(weight, np.float32)

    in_maps = []
    for r in range(M):
        bs, be = r * BL, (r + 1) * BL
        v = np.empty((BA, T, 3, N), dtype=BF16_NP)
        v[:, :, 0, :] = logit[:, bs:be].reshape(T, BA, N).transpose(1, 0, 2)
        v[:, :, 1, :] = q_value[:, bs:be].reshape(T, BA, N).transpose(1, 0, 2)
        v[:, :, 2, :] = (
            target_q_value[:, bs:be].reshape(T, BA, N).transpose(1, 0, 2)
        )
        v = v.reshape(BA, T, 3 * N)
        in_maps.append(
            {
                "v": v,
                "act": np.ascontiguousarray(
                    action[:, bs:be].reshape(T, BA).T.astype(BF16_NP)
                ),
                "wgt": np.ascontiguousarray(weight[:, bs:be].reshape(T, BA).T),
                "rwd": np.ascontiguousarray(
                    np.repeat(reward[:, bs:be], A, axis=1).T
                ),
            }
        )
    return in_maps


def combine_partials(partials_per_core):
    """[M][128,3] partial sums -> the three scalar losses."""
    s = np.stack(partials_per_core).astype(np.float64).sum(axis=(0, 1))
    policy_loss = np.float32(-s[0] / (T * B * A))
    q_value_loss = np.float32(s[1] / ((T - 1) * B * A))
    entropy_loss = np.float32(s[2] / (T * B * A))
    return policy_loss, q_value_loss, entropy_loss


_program_cache = {}


def _get_program() -> bass.Bass:
    if "nc" not in _program_cache:
        nc = build_program()
        nc.finalize()
        _program_cache["nc"] = nc
    return _program_cache["nc"]


def kernel(logit, action, q_value, target_q_value, reward, weight):
    nc = _get_program()
    in_maps = make_in_maps(logit, action, q_value, target_q_value, reward, weight)
    res = run_bass_kernel_spmd(nc, in_maps, list(range(M))).results
    return combine_partials([np.asarray(res[i]["out"]) for i in range(M)])


# revision 12
# speedup vs baseline: 1.0173x; 1.0173x over previous
"""COMA loss kernel for Trainium2 (8 NeuronCores, data-parallel over batch).

Reference computation: given logit/q_value/target_q_value (T,B,A,N),
action (T,B,A), reward (T,B), weight (T,B,A), compute
(policy_loss, q_value_loss, entropy_loss) scalars.

Sharding: B=128 split 8 ways -> B_local=16 per core; the (b,a) plane is
16*8 = 128 rows = the SBUF partition count. Per-core layout is
[BA=128, T, ...], all big tensors cast to bf16 on the host (rel-err
impact measured at ~3e-4 on CPU, far under the 2e-2 gate).

Perf notes (measured on HW, not the cost model):
  * tensor_tensor bf16 SBUF = 2x mode (0.54 ns/elem); scalar_tensor_tensor
    has NO fast uop (1x) and tensor_reduce is 1x -- so all elementwise ops
    are plain tensor_tensor and every per-(t,ba) sum over N is a pairwise
    halving tree of TT adds (2x), merged across the 6 reduced quantities
    into one 6-wide tree to amortize per-op overhead.
  * exp and the onehot comparand broadcast run on the Scalar engine.
  * the three input tensors are packed host-side into one [BA,T,3N] tensor;
    chunk DMAs are split across the SP, Activation and GpSimd(SWDGE) queues
    (one queue sustains only ~114 GB/s; three run concurrently).

Each core emits per-partition partial sums [128,3]; the host adds them
and divides by the global element counts (the all-reduce of the three
scalar means).
"""

import sys

for _p in ("/opt/trn_rl_repo",):
    if _p not in sys.path:
        sys.path.insert(0, _p)

import numpy as np
import ml_dtypes

import concourse.bass as bass
import concourse.bacc as bacc
import concourse.mybir as mybir
from concourse.bass_utils import run_bass_kernel_spmd
from concourse.tile import TileContext

T, B, A, N = 256, 128, 8, 64
M = 8                 # cores
BL = B // M           # local batch
BA = BL * A           # 128 rows -> partition dim
TC = 64               # t-chunk size
NCH = T // TC
GAMMA, LAMBDA = 0.99, 0.95

F32 = mybir.dt.float32
BF16 = mybir.dt.bfloat16
BF16_NP = ml_dtypes.bfloat16

# merged-reduce group order within the 6*N-wide product tile
J_LTK, J_QTK, J_TQTK, J_EL, J_EQ, J_SE = range(6)


def build_program() -> bass.Bass:
    nc = bacc.Bacc("TRN2", target_bir_lowering=False, debug=False)

    # packed [lg | qt | tq] along the last axis
    vd = nc.dram_tensor("v", [BA, T, 3 * N], BF16, kind="ExternalInput")
    act = nc.dram_tensor("act", [BA, T], BF16, kind="ExternalInput")
    wgt = nc.dram_tensor("wgt", [BA, T], F32, kind="ExternalInput")
    rwd = nc.dram_tensor("rwd", [BA, T], F32, kind="ExternalInput")
    out = nc.dram_tensor("out", [BA, 3], F32, kind="ExternalOutput")

    OP = mybir.AluOpType
    AX = mybir.AxisListType
    Act = mybir.ActivationFunctionType

    with TileContext(nc) as tc:
        with (
            tc.tile_pool(name="inp", bufs=2) as inp,
            tc.tile_pool(name="grpp", bufs=1) as grpp,
            tc.tile_pool(name="scr", bufs=1) as scr,
            tc.tile_pool(name="per", bufs=1) as per,
        ):
            # iota over N as bf16 (0..63 exact)
            iota_i = per.tile([BA, N], mybir.dt.int32)
            nc.gpsimd.iota(iota_i[:], pattern=[[1, N]], base=0, channel_multiplier=0)
            iota_f = per.tile([BA, N], BF16)
            nc.vector.tensor_copy(iota_f[:], iota_i[:])

            act_t = per.tile([BA, T], BF16)
            nc.sync.dma_start(out=act_t[:], in_=act[:])
            w_t = per.tile([BA, T], F32)
            nc.sync.dma_start(out=w_t[:], in_=wgt[:])
            r_t = per.tile([BA, T], F32)
            nc.sync.dma_start(out=r_t[:], in_=rwd[:])

            # the 6 per-(t,ba) reductions, interleaved: [BA, T, 6]
            acc6 = per.tile([BA, T, 6], F32)

            for c in range(NCH):
                t0 = c * TC
                sl = slice(t0, t0 + TC)

                v = inp.tile([BA, TC, 3 * N], BF16, tag="v")
                # split each chunk across the two HWDGE DMA queues (SP + Act);
                # one queue alone sustains only ~114 GB/s
                s1 = TC // 2
                nc.sync.dma_start(
                    out=v[:, 0:s1], in_=vd[:, t0 : t0 + s1]
                )
                nc.scalar.dma_start(
                    out=v[:, s1:TC], in_=vd[:, t0 + s1 : t0 + TC]
                )
                lg, qt, tq = (v[:, :, j * N : (j + 1) * N] for j in range(3))

                # onehot comparand: action replicated across N (Scalar engine)
                arep = scr.tile([BA, TC, N], BF16, tag="arep")
                nc.scalar.activation(
                    out=arep[:],
                    in_=act_t[:, sl].unsqueeze(2).to_broadcast([BA, TC, N]),
                    func=Act.Copy,
                )

                # product groups, contiguous so one merged tree reduces all 6:
                # [oh*lg | oh*qt | oh*tq | e*lg | e*qt | e]
                grp = grpp.tile([BA, TC, 6 * N], BF16, tag="grp")
                e = grp[:, :, J_SE * N : (J_SE + 1) * N]
                nc.scalar.activation(out=e, in_=lg, func=Act.Exp)

                oh = scr.tile([BA, TC, N], BF16, tag="oh")
                nc.vector.tensor_tensor(
                    out=oh[:],
                    in0=iota_f[:].unsqueeze(1).to_broadcast([BA, TC, N]),
                    in1=arep[:],
                    op=OP.is_equal,
                )

                for j, sec in ((J_LTK, lg), (J_QTK, qt)):
                    nc.vector.tensor_mul(
                        grp[:, :, j * N : (j + 1) * N], oh[:], sec
                    )
                # the tq gather product runs on the otherwise-idle GpSimd
                nc.gpsimd.tensor_mul(
                    grp[:, :, J_TQTK * N : (J_TQTK + 1) * N], oh[:], tq
                )
                nc.vector.tensor_mul(grp[:, :, J_EL * N : (J_EL + 1) * N], e, lg)
                nc.vector.tensor_mul(grp[:, :, J_EQ * N : (J_EQ + 1) * N], e, qt)

                # merged pairwise tree over the last-axis width N for all 6
                # groups at once; 4-D views [BA, TC, 6, w]
                cur, w = grp[:].rearrange("p t (g n) -> p t g n", g=6), N
                while w > 2:
                    h = w // 2
                    nxt = scr.tile([BA, TC, 6, h], BF16, tag=f"tr{h}")
                    nc.vector.tensor_add(
                        nxt[:], cur[:, :, :, 0:h], cur[:, :, :, h:w]
                    )
                    cur, w = nxt, h
                nc.vector.tensor_add(
                    acc6[:, sl].unsqueeze(3),
                    cur[:, :, :, 0:1],
                    cur[:, :, :, 1:2],
                )

            # ---- stage 2: per-(t,ba) scalar math on [128, T] -------------
            l_tk = acc6[:, :, J_LTK]
            q_tk = acc6[:, :, J_QTK]
            tq_tk = acc6[:, :, J_TQTK]
            dot_el = acc6[:, :, J_EL]
            dot_eq = acc6[:, :, J_EQ]
            sum_e = acc6[:, :, J_SE]

            z = per.tile([BA, T], F32)  # logsumexp
            nc.scalar.activation(out=z[:], in_=sum_e, func=Act.Ln)
            rs = per.tile([BA, T], F32)  # 1/sum_e
            nc.vector.reciprocal(rs[:], sum_e)

            logp = per.tile([BA, T], F32)
            nc.vector.tensor_tensor(out=logp[:], in0=l_tk, in1=z[:], op=OP.subtract)
            bl = per.tile([BA, T], F32)  # baseline = dot_eq / sum_e
            nc.vector.tensor_mul(bl[:], dot_eq, rs[:])
            adv = per.tile([BA, T], F32)
            nc.vector.tensor_tensor(out=adv[:], in0=q_tk, in1=bl[:], op=OP.subtract)
            ent = per.tile([BA, T], F32)  # entropy = z - dot_el / sum_e
            nc.vector.tensor_mul(ent[:], dot_el, rs[:])
            nc.vector.tensor_tensor(out=ent[:], in0=z[:], in1=ent[:], op=OP.subtract)

            pol = per.tile([BA, T], F32)  # logp * adv * w
            nc.vector.tensor_mul(pol[:], logp[:], adv[:])
            nc.vector.tensor_mul(pol[:], pol[:], w_t[:])
            entw = per.tile([BA, T], F32)
            nc.vector.tensor_mul(entw[:], ent[:], w_t[:])

            # lambda returns: ret[t] = d[t] + g*l*ret[t+1], scanned in
            # reverse time via negative-step views.
            # d[t] = reward[t] + g*(1-l)*tq_taken[t+1], t in [0, T-2];
            # initial state tq_taken[T-1] makes ret[T-2] = reward[T-2] +
            # g*tq_taken[T-1] as required.
            d = per.tile([BA, T - 1], F32)
            nc.vector.tensor_scalar_mul(d[:], tq_tk[:, 1:T], GAMMA * (1.0 - LAMBDA))
            nc.vector.tensor_add(d[:], d[:], r_t[:, 0 : T - 1])
            gl = per.tile([BA, 1], F32)
            nc.vector.memset(gl[:], GAMMA * LAMBDA)
            ret = per.tile([BA, T - 1], F32)
            nc.vector.tensor_tensor_scan(
                out=ret[:, ::-1],
                data0=gl[:].to_broadcast([BA, T - 1]),
                data1=d[:, ::-1],
                initial=tq_tk[:, T - 1 : T],
                op0=OP.mult,
                op1=OP.add,
            )

            qd = per.tile([BA, T - 1], F32)
            nc.vector.tensor_tensor(
                out=qd[:], in0=ret[:], in1=q_tk[:, 0 : T - 1], op=OP.subtract
            )
            nc.vector.tensor_mul(qd[:], qd[:], qd[:])
            nc.vector.tensor_mul(qd[:], qd[:], w_t[:, 0 : T - 1])

            partials = per.tile([BA, 3], F32)
            nc.vector.reduce_sum(out=partials[:, 0:1], in_=pol[:], axis=AX.X)
            nc.vector.reduce_sum(out=partials[:, 1:2], in_=qd[:], axis=AX.X)
            nc.vector.reduce_sum(out=partials[:, 2:3], in_=entw[:], axis=AX.X)
            nc.sync.dma_start(out=out[:], in_=partials[:])

    return nc


def make_in_maps(logit, action, q_value, target_q_value, reward, weight):
    """Shard + marshal full inputs into per-core input dicts."""
    logit = np.asarray(logit, np.float32)
    q_value = np.asarray(q_value, np.float32)
    target_q_value = np.asarray(target_q_value, np.float32)
    action = np.asarray(action)
    reward = np.asarray(reward, np.float32)
    weight = np.asarray(weight, np.float32)

    in_maps = []
    for r in range(M):
        bs, be = r * BL, (r + 1) * BL
        v = np.empty((BA, T, 3, N), dtype=BF16_NP)
        v[:, :, 0, :] = logit[:, bs:be].reshape(T, BA, N).transpose(1, 0, 2)
        v[:, :, 1, :] = q_value[:, bs:be].reshape(T, BA, N).transpose(1, 0, 2)
        v[:, :, 2, :] = (
            target_q_value[:, bs:be].reshape(T, BA, N).transpose(1, 0, 2)
        )
        v = v.reshape(BA, T, 3 * N)
        in_maps.append(
            {
                "v": v,
                "act": np.ascontiguousarray(
                    action[:, bs:be].reshape(T, BA).T.astype(BF16_NP)
                ),
                "wgt": np.ascontiguousarray(weight[:, bs:be].reshape(T, BA).T),
                "rwd": np.ascontiguousarray(
                    np.repeat(reward[:, bs:be], A, axis=1).T
                ),
            }
        )
    return in_maps


def combine_partials(partials_per_core):
    """[M][128,3] partial sums -> the three scalar losses."""
    s = np.stack(partials_per_core).astype(np.float64).sum(axis=(0, 1))
    policy_loss = np.float32(-s[0] / (T * B * A))
    q_value_loss = np.float32(s[1] / ((T - 1) * B * A))
    entropy_loss = np.float32(s[2] / (T * B * A))
    return policy_loss, q_value_loss, entropy_loss


_program_cache = {}


def _get_program() -> bass.Bass:
    if "nc" not in _program_cache:
        nc = build_program()
        nc.finalize()
        _program_cache["nc"] = nc
    return _program_cache["nc"]


def kernel(logit, action, q_value, target_q_value, reward, weight):
    nc = _get_program()
    in_maps = make_in_maps(logit, action, q_value, target_q_value, reward, weight)
    res = run_bass_kernel_spmd(nc, in_maps, list(range(M))).results
    return combine_partials([np.asarray(res[i]["out"]) for i in range(M)])


# revision 13
# speedup vs baseline: 1.1493x; 1.1297x over previous
"""COMA loss kernel for Trainium2 (8 NeuronCores, data-parallel over batch).

Reference computation: given logit/q_value/target_q_value (T,B,A,N),
action (T,B,A), reward (T,B), weight (T,B,A), compute
(policy_loss, q_value_loss, entropy_loss) scalars.

Sharding: B=128 split 8 ways -> B_local=16 per core; the (b,a) plane is
16*8 = 128 rows = the SBUF partition count. Per-core layout is
[BA=128, T, ...], all big tensors cast to bf16 on the host (rel-err
impact measured at ~3e-4 on CPU, far under the 2e-2 gate).

Perf notes (measured on HW, not the cost model):
  * tensor_tensor bf16 SBUF = 2x mode (0.54 ns/elem); scalar_tensor_tensor
    has NO fast uop (1x) and tensor_reduce is 1x -- so all elementwise ops
    are plain tensor_tensor and every per-(t,ba) sum over N is a pairwise
    halving tree of TT adds (2x), merged across the 6 reduced quantities
    into one 6-wide tree to amortize per-op overhead.
  * exp and the onehot comparand broadcast run on the Scalar engine.
  * the three input tensors are packed host-side into one [BA,T,3N] tensor;
    chunk DMAs are split across the SP, Activation and GpSimd(SWDGE) queues
    (one queue sustains only ~114 GB/s; three run concurrently).

Each core emits per-partition partial sums [128,3]; the host adds them
and divides by the global element counts (the all-reduce of the three
scalar means).
"""

import sys

for _p in ("/opt/trn_rl_repo",):
    if _p not in sys.path:
        sys.path.insert(0, _p)

import numpy as np
import ml_dtypes

import concourse.bass as bass
import concourse.bacc as bacc
import concourse.mybir as mybir
from concourse.bass_utils import run_bass_kernel_spmd
from concourse.tile import TileContext

T, B, A, N = 256, 128, 8, 64
M = 8                 # cores
BL = B // M           # local batch
BA = BL * A           # 128 rows -> partition dim
TC = 64               # t-chunk size
NCH = T // TC
GAMMA, LAMBDA = 0.99, 0.95

F32 = mybir.dt.float32
BF16 = mybir.dt.bfloat16
BF16_NP = ml_dtypes.bfloat16

# merged-reduce group order within the 6*N-wide product tile
J_LTK, J_QTK, J_TQTK, J_EL, J_EQ, J_SE = range(6)


def build_program() -> bass.Bass:
    nc = bacc.Bacc("TRN2", target_bir_lowering=False, debug=False)

    # packed [lg | qt | tq] along the last axis
    vd = nc.dram_tensor("v", [BA, T, 3 * N], BF16, kind="ExternalInput")
    act = nc.dram_tensor("act", [BA, T], BF16, kind="ExternalInput")
    wgt = nc.dram_tensor("wgt", [BA, T], F32, kind="ExternalInput")
    rwd = nc.dram_tensor("rwd", [BA, T], F32, kind="ExternalInput")
    out = nc.dram_tensor("out", [BA, 3], F32, kind="ExternalOutput")

    OP = mybir.AluOpType
    AX = mybir.AxisListType
    Act = mybir.ActivationFunctionType

    with TileContext(nc) as tc:
        with (
            tc.tile_pool(name="inp", bufs=2) as inp,
            tc.tile_pool(name="scr", bufs=1) as scr,
            tc.tile_pool(name="per", bufs=1) as per,
        ):
            # iota over N as bf16 (0..63 exact)
            iota_i = per.tile([BA, N], mybir.dt.int32)
            nc.gpsimd.iota(iota_i[:], pattern=[[1, N]], base=0, channel_multiplier=0)
            iota_f = per.tile([BA, N], BF16)
            nc.vector.tensor_copy(iota_f[:], iota_i[:])

            act_t = per.tile([BA, T], BF16)
            nc.sync.dma_start(out=act_t[:], in_=act[:])
            w_t = per.tile([BA, T], F32)
            nc.sync.dma_start(out=w_t[:], in_=wgt[:])
            r_t = per.tile([BA, T], F32)
            nc.sync.dma_start(out=r_t[:], in_=rwd[:])

            # the 6 per-(t,ba) reductions, interleaved: [BA, T, 6]
            acc6 = per.tile([BA, T, 6], F32)

            for c in range(NCH):
                t0 = c * TC
                sl = slice(t0, t0 + TC)

                v = inp.tile([BA, TC, 3 * N], BF16, tag="v")
                # split each chunk across the three DMA queues
                s1, s2 = TC // 3, 2 * TC // 3
                nc.sync.dma_start(
                    out=v[:, 0:s1], in_=vd[:, t0 : t0 + s1]
                )
                nc.scalar.dma_start(
                    out=v[:, s1:s2], in_=vd[:, t0 + s1 : t0 + s2]
                )
                nc.gpsimd.dma_start(
                    out=v[:, s2:TC], in_=vd[:, t0 + s2 : t0 + TC]
                )
                lg, qt, tq = (v[:, :, j * N : (j + 1) * N] for j in range(3))

                # onehot comparand: action replicated across N (Scalar engine)
                arep = scr.tile([BA, TC, N], BF16, tag="arep")
                nc.scalar.activation(
                    out=arep[:],
                    in_=act_t[:, sl].unsqueeze(2).to_broadcast([BA, TC, N]),
                    func=Act.Copy,
                )

                # product groups, contiguous so one merged tree reduces all 6:
                # [oh*lg | oh*qt | oh*tq | e*lg | e*qt | e]
                grp = scr.tile([BA, TC, 6 * N], BF16, tag="grp")
                e = grp[:, :, J_SE * N : (J_SE + 1) * N]
                nc.scalar.activation(out=e, in_=lg, func=Act.Exp)

                oh = scr.tile([BA, TC, N], BF16, tag="oh")
                nc.vector.tensor_tensor(
                    out=oh[:],
                    in0=iota_f[:].unsqueeze(1).to_broadcast([BA, TC, N]),
                    in1=arep[:],
                    op=OP.is_equal,
                )

                for j, sec in ((J_LTK, lg), (J_QTK, qt), (J_TQTK, tq)):
                    nc.vector.tensor_mul(
                        grp[:, :, j * N : (j + 1) * N], oh[:], sec
                    )
                nc.vector.tensor_mul(grp[:, :, J_EL * N : (J_EL + 1) * N], e, lg)
                nc.vector.tensor_mul(grp[:, :, J_EQ * N : (J_EQ + 1) * N], e, qt)

                # merged pairwise tree over the last-axis width N for all 6
                # groups at once; 4-D views [BA, TC, 6, w]
                cur, w = grp[:].rearrange("p t (g n) -> p t g n", g=6), N
                while w > 2:
                    h = w // 2
                    nxt = scr.tile([BA, TC, 6, h], BF16, tag=f"tr{h}")
                    nc.vector.tensor_add(
                        nxt[:], cur[:, :, :, 0:h], cur[:, :, :, h:w]
                    )
                    cur, w = nxt, h
                nc.vector.tensor_add(
                    acc6[:, sl].unsqueeze(3),
                    cur[:, :, :, 0:1],
                    cur[:, :, :, 1:2],
                )

            # ---- stage 2: per-(t,ba) scalar math on [128, T] -------------
            l_tk = acc6[:, :, J_LTK]
            q_tk = acc6[:, :, J_QTK]
            tq_tk = acc6[:, :, J_TQTK]
            dot_el = acc6[:, :, J_EL]
            dot_eq = acc6[:, :, J_EQ]
            sum_e = acc6[:, :, J_SE]

            z = per.tile([BA, T], F32)  # logsumexp
            nc.scalar.activation(out=z[:], in_=sum_e, func=Act.Ln)
            rs = per.tile([BA, T], F32)  # 1/sum_e
            nc.vector.reciprocal(rs[:], sum_e)

            logp = per.tile([BA, T], F32)
            nc.vector.tensor_tensor(out=logp[:], in0=l_tk, in1=z[:], op=OP.subtract)
            bl = per.tile([BA, T], F32)  # baseline = dot_eq / sum_e
            nc.vector.tensor_mul(bl[:], dot_eq, rs[:])
            adv = per.tile([BA, T], F32)
            nc.vector.tensor_tensor(out=adv[:], in0=q_tk, in1=bl[:], op=OP.subtract)
            ent = per.tile([BA, T], F32)  # entropy = z - dot_el / sum_e
            nc.vector.tensor_mul(ent[:], dot_el, rs[:])
            nc.vector.tensor_tensor(out=ent[:], in0=z[:], in1=ent[:], op=OP.subtract)

            pol = per.tile([BA, T], F32)  # logp * adv * w
            nc.vector.tensor_mul(pol[:], logp[:], adv[:])
            nc.vector.tensor_mul(pol[:], pol[:], w_t[:])
            entw = per.tile([BA, T], F32)
            nc.vector.tensor_mul(entw[:], ent[:], w_t[:])

            # lambda returns: ret[t] = d[t] + g*l*ret[t+1], scanned in
            # reverse time via negative-step views.
            # d[t] = reward[t] + g*(1-l)*tq_taken[t+1], t in [0, T-2];
            # initial state tq_taken[T-1] makes ret[T-2] = reward[T-2] +
            # g*tq_taken[T-1] as required.
            d = per.tile([BA, T - 1], F32)
            nc.vector.tensor_scalar_mul(d[:], tq_tk[:, 1:T], GAMMA * (1.0 - LAMBDA))
            nc.vector.tensor_add(d[:], d[:], r_t[:, 0 : T - 1])
            gl = per.tile([BA, 1], F32)
            nc.vector.memset(gl[:], GAMMA * LAMBDA)
            ret = per.tile([BA, T - 1], F32)
            nc.vector.tensor_tensor_scan(
                out=ret[:, ::-1],
                data0=gl[:].to_broadcast([BA, T - 1]),
                data1=d[:, ::-1],
                initial=tq_tk[:, T - 1 : T],
                op0=OP.mult,
                op1=OP.add,
            )

            qd = per.tile([BA, T - 1], F32)
            nc.vector.tensor_tensor(
                out=qd[:], in0=ret[:], in1=q_tk[:, 0 : T - 1], op=OP.subtract
            )
            nc.vector.tensor_mul(qd[:], qd[:], qd[:])
            nc.vector.tensor_mul(qd[:], qd[:], w_t[:, 0 : T - 1])

            partials = per.tile([BA, 3], F32)
            nc.vector.reduce_sum(out=partials[:, 0:1], in_=pol[:], axis=AX.X)
            nc.vector.reduce_sum(out=partials[:, 1:2], in_=qd[:], axis=AX.X)
            nc.vector.reduce_sum(out=partials[:, 2:3], in_=entw[:], axis=AX.X)
            nc.sync.dma_start(out=out[:], in_=partials[:])

    return nc


def make_in_maps(logit, action, q_value, target_q_value, reward, weight):
    """Shard + marshal full inputs into per-core input dicts."""
    logit = np.asarray(logit, np.float32)
    q_value = np.asarray(q_value, np.float32)
    target_q_value = np.asarray(target_q_value, np.float32)
    action = np.asarray(action)
    reward = np.asarray(reward, np.float32)
    weight = np.asarray(weight, np.float32)

    in_maps = []
    for r in range(M):
        bs, be = r * BL, (r + 1) * BL
        v = np.empty((BA, T, 3, N), dtype=BF16_NP)
        v[:, :, 0, :] = logit[:, bs:be].reshape(T, BA, N).transpose(1, 0, 2)
        v[:, :, 1, :] = q_value[:, bs:be].reshape(T, BA, N).transpose(1, 0, 2)
        v[:, :, 2, :] = (
            target_q_value[:, bs:be].reshape(T, BA, N).transpose(1, 0, 2)
        )
        v = v.reshape(BA, T, 3 * N)
        in_maps.append(
            {
                "v": v,
                "act": np.ascontiguousarray(
                    action[:, bs:be].reshape(T, BA).T.astype(BF16_NP)
                ),
                "wgt": np.ascontiguousarray(weight[:, bs:be].reshape(T, BA).T),
                "rwd": np.ascontiguousarray(
                    np.repeat(reward[:, bs:be], A, axis=1).T
                ),
            }
        )
    return in_maps


def combine_partials(partials_per_core):
    """[M][128,3] partial sums -> the three scalar losses."""
    s = np.stack(partials_per_core).astype(np.float64).sum(axis=(0, 1))
    policy_loss = np.float32(-s[0] / (T * B * A))
    q_value_loss = np.float32(s[1] / ((T - 1) * B * A))
    entropy_loss = np.float32(s[2] / (T * B * A))
    return policy_loss, q_value_loss, entropy_loss


_program_cache = {}


def _get_program() -> bass.Bass:
    if "nc" not in _program_cache:
        nc = build_program()
        nc.finalize()
        _program_cache["nc"] = nc
    return _program_cache["nc"]


def kernel(logit, action, q_value, target_q_value, reward, weight):
    nc = _get_program()
    in_maps = make_in_maps(logit, action, q_value, target_q_value, reward, weight)
    res = run_bass_kernel_spmd(nc, in_maps, list(range(M))).results
    return combine_partials([np.asarray(res[i]["out"]) for i in range(M)])
